# revision 1
# baseline (speedup 1.0000x reference)
"""Single-head causal self-attention on 8 NeuronCores (data-parallel over batch).

Reference computation (per batch element b):
    Q = X @ Wq + bq; K = X @ Wk + bk; V = X @ Wv + bv        # [T, DK]
    S = Q @ K.T / sqrt(DK)  (causal masked)
    out = softmax(S) @ V                                      # [T, DK]

Device strategy (one batch element per core):
  - Host passes X.T [C, T] so every DMA row is contiguous.
  - Two full-width projection passes with packed stationaries:
      pass A: [Wv | Wk] -> psum rows 0:64 = V.T, rows 64:128 = K.T
      pass B: [Wq | Wq] -> Q.T duplicated in both partition halves
    Biases are added exactly during the PSUM->SBUF drain (tensor_scalar_add
    with a per-partition vector).
  - V.T is PE-transposed into natural [s, dk] tiles with a ones column
    appended; the ones column makes the output matmul also produce the
    softmax denominator l (row 64 of the output).
  - Scores are computed transposed: S.T[s, t] = K.T^T @ Q.T, so softmax's
    exp (ScalarE, scale=1/8 fused) and the P@V contraction (over s = the
    partition dim) need no large transposes. Causality = skip tiles below
    the diagonal + one upper-triangular 128x128 mask multiply per s-tile.
  - Device output per core: [65, T] = rows 0:64 unnormalized O.T, row 64 l.
    Host computes (O_unnorm / l).T.
"""

import sys

sys.path.insert(0, "/opt/trn_rl_repo")

import numpy as np

B, T, C, DK = 8, 2048, 1024, 64
KT = C // 128          # 8 k-tiles in the contraction over C
NS = T // 128          # 16 s-tiles (key blocks)
NCHUNK = T // 512      # 4 output chunks of 512
SCALE = 1.0 / np.sqrt(DK)

_CACHE = {}


def _build():
    from concourse import bass, bacc, tile

    mybir = bass.mybir
    f32 = mybir.dt.float32
    f32r = mybir.dt.float32r

    nc = bacc.Bacc(
        "TRN2", target_bir_lowering=False, debug=False, num_devices=B
    )

    xt_d = nc.dram_tensor("xt", [KT, 128, T], f32r, kind="ExternalInput")
    wvk_d = nc.dram_tensor("wvk", [128, KT * 128], f32r, kind="ExternalInput")
    wqq_d = nc.dram_tensor("wqq", [128, KT * 128], f32r, kind="ExternalInput")
    bvk_d = nc.dram_tensor("bvk", [128, 1], f32, kind="ExternalInput")
    bqq_d = nc.dram_tensor("bqq", [128, 1], f32, kind="ExternalInput")
    out_d = nc.dram_tensor("out", [65, T], f32, kind="ExternalOutput")

    # one packed const block: cols 0:128 tri-mask, 128:192 ident (rows 0:64),
    # col 192 reserved
    cst_np = np.zeros((128, 193), dtype=np.float32)
    cst_np[:, 0:128] = np.triu(np.ones((128, 128), dtype=np.float32))
    cst_np[0:64, 128:192] = np.eye(64, dtype=np.float32)
    cst_d = nc.inline_tensor(cst_np, "cst")

    EXP = mybir.ActivationFunctionType.Exp

    with tile.TileContext(nc) as tc:
        with tc.tile_pool(name="const", bufs=1) as cpool, \
             tc.tile_pool(name="weights", bufs=1) as wpool, \
             tc.tile_pool(name="x", bufs=1) as xpool, \
             tc.tile_pool(name="acts", bufs=1) as apool:

            cst = cpool.tile([128, 193], f32r)
            nc.gpsimd.dma_start(out=cst[:], in_=cst_d[:].bitcast(f32r))
            tri = cst[:, 0:128]
            ident = cst[0:64, 128:192]
            bvk = cpool.tile([128, 1], f32)
            nc.gpsimd.dma_start(out=bvk[:], in_=bvk_d[:])
            bqq = cpool.tile([128, 1], f32)
            nc.gpsimd.dma_start(out=bqq[:], in_=bqq_d[:])

            wvk = wpool.tile([128, KT * 128], f32r)
            wqq = wpool.tile([128, KT * 128], f32r)
            nc.scalar.dma_start(out=wvk[:], in_=wvk_d[:])
            nc.scalar.dma_start(out=wqq[:], in_=wqq_d[:])

            dma_engs = [nc.sync, nc.gpsimd, nc.scalar]
            xts = []
            for k in range(KT):
                xk = xpool.tile([128, T], f32r, tag=f"x{k}")
                dma_engs[k % 3].dma_start(out=xk[:], in_=xt_d[k])
                xts.append(xk)

            # persistent activations
            vk = apool.tile([128, T], f32r, tag="vk")    # V.T rows 0:64, K.T rows 64:128
            qq = apool.tile([128, T], f32r, tag="qq")    # Q.T in both halves
            v1 = apool.tile([128, NS * 65], f32r, tag="v1")  # [V_i | 1] stationaries
            osb = apool.tile([65, T], f32, tag="osb")

            nc.gpsimd.memset(v1[:].bitcast(f32), 1.0)

            # ---------------- projections ----------------
            with tc.tile_pool(name="pproj", bufs=1, space="PSUM") as pproj:
                psA = pproj.tile([128, T], f32, tag="psA")
                psB = pproj.tile([128, T], f32, tag="psB")
                for k in range(KT):
                    for c in range(NCHUNK):
                        sl = slice(512 * c, 512 * (c + 1))
                        nc.tensor.matmul(
                            psA[:, sl],
                            wvk[:, 128 * k:128 * (k + 1)],
                            xts[k][:, sl],
                            start=(k == 0), stop=(k == KT - 1),
                        )
                    for c in range(NCHUNK):
                        sl = slice(512 * c, 512 * (c + 1))
                        nc.tensor.matmul(
                            psB[:, sl],
                            wqq[:, 128 * k:128 * (k + 1)],
                            xts[k][:, sl],
                            start=(k == 0), stop=(k == KT - 1),
                        )
                # drain with exact bias add, pipelined per 512-chunk
                for c in range(NCHUNK):
                    sl = slice(512 * c, 512 * (c + 1))
                    nc.vector.tensor_scalar_add(vk[:, sl], psA[:, sl], bvk[:])
                    nc.vector.tensor_scalar_add(qq[:, sl], psB[:, sl], bqq[:])

            # ---------------- V transposes ----------------
            with tc.tile_pool(name="pv", bufs=2, space="PSUM") as pv:
                for i in range(NS):
                    vt = pv.tile([128, 64], f32r, tag="vt")
                    nc.tensor.transpose(
                        vt[:], vk[0:64, 128 * i:128 * (i + 1)], ident[:]
                    )
                    nc.vector.tensor_copy(v1[:, 65 * i:65 * i + 64], vt[:])

            # ---------------- attention ----------------
            with tc.tile_pool(name="po", bufs=1, space="PSUM") as po, \
                 tc.tile_pool(name="pst", bufs=2, space="PSUM") as pst, \
                 tc.tile_pool(name="et", bufs=3) as etpool:

                ops = [
                    po.tile([65, 512], f32, tag=f"o{j}", name=f"o{j}")
                    for j in range(NCHUNK)
                ]

                for i in range(NS):
                    ts = 128 * i
                    jmin = i // 4
                    et = etpool.tile([128, T], f32r, tag="et")
                    if ts > 512 * jmin:
                        nc.gpsimd.memset(et[:, 512 * jmin:ts].bitcast(f32), 0.0)
                    for tb in range(ts // 1024, 2):
                        st = pst.tile([128, 1024], f32, tag="st")
                        for cc in range(2):
                            t0 = 1024 * tb + 512 * cc
                            if t0 + 512 <= ts:
                                continue
                            nc.tensor.matmul(
                                st[:, 512 * cc:512 * (cc + 1)],
                                vk[64:128, 128 * i:128 * (i + 1)],
                                qq[64:128, t0:t0 + 512],
                                start=True, stop=True,
                            )
                        off = max(0, ts - 1024 * tb)
                        nc.scalar.activation(
                            et[:, 1024 * tb + off:1024 * (tb + 1)],
                            st[:, off:1024],
                            EXP, scale=SCALE,
                        )
                    # causal mask on the diagonal 128-block
                    nc.vector.tensor_mul(
                        et[:, ts:ts + 128], et[:, ts:ts + 128], tri[:]
                    )
                    for j in range(jmin, NCHUNK):
                        nc.tensor.matmul(
                            ops[j][:],
                            v1[:, 65 * i:65 * i + 65],
                            et[:, 512 * j:512 * (j + 1)],
                            start=(i == 0), stop=(i == 4 * j + 3),
                        )
                    # drain any output chunk whose accumulation just finished
                    for j in range(jmin, NCHUNK):
                        if i == 4 * j + 3:
                            sl = slice(512 * j, 512 * (j + 1))
                            nc.vector.tensor_copy(osb[:, sl], ops[j][:])
                            nc.sync.dma_start(out=out_d[:, sl], in_=osb[:, sl])

    nc.compile()
    return nc


def _get_nc():
    if "nc" not in _CACHE:
        _CACHE["nc"] = _build()
    return _CACHE["nc"]


def make_in_maps(X, Wq, bq, Wk, bk, Wv, bv):
    X = np.asarray(X, dtype=np.float32)
    Wq = np.asarray(Wq, dtype=np.float32)
    Wk = np.asarray(Wk, dtype=np.float32)
    Wv = np.asarray(Wv, dtype=np.float32)
    bq = np.asarray(bq, dtype=np.float32)
    bk = np.asarray(bk, dtype=np.float32)
    bv = np.asarray(bv, dtype=np.float32)

    wvk = np.ascontiguousarray(
        np.concatenate([Wv, Wk], axis=1).reshape(KT, 128, 128)
        .transpose(1, 0, 2).reshape(128, KT * 128)
    )
    wqq = np.ascontiguousarray(
        np.concatenate([Wq, Wq], axis=1).reshape(KT, 128, 128)
        .transpose(1, 0, 2).reshape(128, KT * 128)
    )
    bvk = np.concatenate([bv, bk]).reshape(128, 1).astype(np.float32)
    bqq = np.concatenate([bq, bq]).reshape(128, 1).astype(np.float32)

    in_maps = []
    for b in range(B):
        xt = np.ascontiguousarray(X[b].T).reshape(KT, 128, T)
        in_maps.append(
            {"xt": xt, "wvk": wvk, "wqq": wqq, "bvk": bvk, "bqq": bqq}
        )
    return in_maps


def kernel(X, Wq, bq, Wk, bk, Wv, bv):
    from concourse.bass_utils import run_bass_kernel_spmd

    nc = _get_nc()
    in_maps = make_in_maps(X, Wq, bq, Wk, bk, Wv, bv)
    res = run_bass_kernel_spmd(nc, in_maps, list(range(B)))

    out = np.empty((B, T, DK), dtype=np.float32)
    for b in range(B):
        r = res.results[b]["out"]
        out[b] = (r[:64] / r[64:65]).T
    return out



# revision 8
# speedup vs baseline: 1.4439x; 1.4439x over previous
"""Single-head causal self-attention on 8 NeuronCores (data-parallel over batch).

Reference computation (per batch element b):
    Q = X @ Wq + bq; K = X @ Wk + bk; V = X @ Wv + bv        # [T, DK]
    S = Q @ K.T / sqrt(DK)  (causal masked)
    out = softmax(S) @ V                                      # [T, DK]

Device strategy (one batch element per core), v2 (bf16):
  - Host passes X.T tiles and packed weights in bf16 (halves HBM traffic
    and SBUF pressure; PSUM accumulation stays fp32).
  - Pass A stationary packs [Wk | Wv] per 128-row C-chunk, so the psum
    holds K.T in partitions 0:64 and V.T in partitions 64:128. Pass B is
    just Wq -> Q.T in a [64, T] psum (no duplication).
  - X tiles stream over three DMA queues in k order; weights/constants go
    on the vector engine's queue so x0 is never stuck behind them.
  - k=0..6 accumulate; the k=7 chunk matmuls are emitted per 512-column
    chunk with the drain of that chunk right behind them, split across
    vector (vk even chunks), gpsimd (vk odd chunks), and scalar (qq with
    exact bias via Identity activation).
  - V.T tiles are PE-transposed into [s, dk] with a ones column appended
    (l = softmax denominator falls out of the output matmul, row 64).
  - Scores are computed transposed, S.T[s, t] = K.T^T @ Q.T, trimmed to
    the causal boundary at 128-column granularity; exp runs on scalar
    (scale=1/8 fused) writing bf16; causality = one triangular mask
    multiply per diagonal block (vector) + trimmed matmuls (no memsets).
  - The P@V contraction accumulates the four 512-wide output chunks in
    psum; per-chunk drain + output DMA as soon as its accumulation ends.
  - PE stream is software-pipelined: scores(i+1) is emitted before PV(i)
    so exp(i) overlaps matmuls; V transposes fill the drain window.
  - Device output per core: [65, T] = rows 0:64 unnormalized O.T, row 64
    l. Host computes (O_unnorm / l).T.
"""

import sys

sys.path.insert(0, "/opt/trn_rl_repo")

import numpy as np
import ml_dtypes

B, T, C, DK = 8, 2048, 1024, 64
KT = C // 128          # 8 k-tiles in the contraction over C
NS = T // 128          # 16 s-tiles (key blocks)
NCHUNK = T // 512      # 4 output chunks of 512
SCALE = 1.0 / np.sqrt(DK)
BF16 = ml_dtypes.bfloat16

_CACHE = {}


def _build():
    from concourse import bass, bacc, tile

    mybir = bass.mybir
    f32 = mybir.dt.float32
    bf16 = mybir.dt.bfloat16

    nc = bacc.Bacc(
        "TRN2", target_bir_lowering=False, debug=False, num_devices=B
    )

    xt_d = nc.dram_tensor("xt", [KT, 128, T], bf16, kind="ExternalInput")
    wkv_d = nc.dram_tensor("wkv", [128, KT * 128], bf16, kind="ExternalInput")
    wq_d = nc.dram_tensor("wq", [128, KT * 64], bf16, kind="ExternalInput")
    bkv_d = nc.dram_tensor("bkv", [128, 1], f32, kind="ExternalInput")
    bq_d = nc.dram_tensor("bq", [64, 1], f32, kind="ExternalInput")
    # cols 0:128 upper-tri mask; cols 128:192 identity (rows 64:128)
    cst_d = nc.dram_tensor("cst", [128, 192], bf16, kind="ExternalInput")
    out_d = nc.dram_tensor("out", [65, T], f32, kind="ExternalOutput")

    EXP = mybir.ActivationFunctionType.Exp
    IDENT = mybir.ActivationFunctionType.Identity

    with tile.TileContext(nc) as tc:
        with tc.tile_pool(name="const", bufs=1) as cpool, \
             tc.tile_pool(name="weights", bufs=1) as wpool, \
             tc.tile_pool(name="x", bufs=1) as xpool, \
             tc.tile_pool(name="acts", bufs=1) as apool:

            # --- inbound DMAs: weights/consts on vector's queue, X tiles
            # --- striped over sync/gpsimd/scalar in k order
            wkv = wpool.tile([128, KT * 128], bf16)
            wq = wpool.tile([128, KT * 64], bf16)
            cst = cpool.tile([128, 192], bf16)
            bkv = cpool.tile([128, 1], f32)
            bq = cpool.tile([64, 1], f32)
            nc.scalar.dma_start(out=wkv[:], in_=wkv_d[:])
            nc.scalar.dma_start(out=wq[:], in_=wq_d[:])
            nc.scalar.dma_start(out=cst[:], in_=cst_d[:])
            nc.scalar.dma_start(out=bkv[:], in_=bkv_d[:])
            nc.scalar.dma_start(out=bq[:], in_=bq_d[:])
            tri = cst[:, 0:128]
            ident64 = cst[64:128, 128:192]

            # x0/x2/x4/x6 on sync (hw dge), x1/x5 on scalar behind the
            # small weight transfers, x3/x7 on gpsimd
            xdma = {0: nc.sync, 2: nc.sync, 4: nc.sync, 6: nc.sync,
                    1: nc.scalar, 5: nc.scalar, 3: nc.gpsimd, 7: nc.gpsimd}
            xts = []
            for k in range(KT):
                xk = xpool.tile([128, T], bf16, tag=f"x{k}")
                xts.append(xk)
            for k in [0, 1, 3, 2, 5, 7, 4, 6]:
                xdma[k].dma_start(out=xts[k][:], in_=xt_d[k])

            # persistent activations
            vk = apool.tile([128, T], bf16, tag="vk")   # K.T rows 0:64, V.T rows 64:128
            qq = apool.tile([64, T], bf16, tag="qq")    # Q.T
            v1 = apool.tile([128, NS * 65], bf16, tag="v1")  # [V_i | 1] stationaries
            osb = apool.tile([65, T], f32, tag="osb")

            nc.gpsimd.memset(v1[:], 1.0)

            # ---------------- projections ----------------
            with tc.tile_pool(name="pproj", bufs=1, space="PSUM") as pproj:
                psA = pproj.tile([128, T], f32, tag="psA")
                psB = pproj.tile([64, T], f32, tag="psB")
                for k in range(KT - 1):
                    for c in range(NCHUNK):
                        sl = slice(512 * c, 512 * (c + 1))
                        nc.tensor.matmul(
                            psA[:, sl],
                            wkv[:, 128 * k:128 * (k + 1)],
                            xts[k][:, sl],
                            start=(k == 0), stop=False,
                        )
                    for c in range(NCHUNK):
                        sl = slice(512 * c, 512 * (c + 1))
                        nc.tensor.matmul(
                            psB[:, sl],
                            wq[:, 64 * k:64 * (k + 1)],
                            xts[k][:, sl],
                            start=(k == 0), stop=False,
                        )
                # k=7 per chunk; qq drains right behind each chunk (qq
                # gates scores(0)), split scalar/vector; vk drains follow
                k = KT - 1
                for c in range(NCHUNK):
                    sl = slice(512 * c, 512 * (c + 1))
                    nc.tensor.matmul(
                        psA[:, sl], wkv[:, 128 * k:128 * (k + 1)],
                        xts[k][:, sl], start=False, stop=True,
                    )
                    nc.tensor.matmul(
                        psB[:, sl], wq[:, 64 * k:64 * (k + 1)],
                        xts[k][:, sl], start=False, stop=True,
                    )
                    if c % 2 == 0:
                        nc.scalar.activation(
                            qq[:, sl], psB[:, sl], IDENT, bias=bq[:]
                        )
                    else:
                        nc.vector.tensor_scalar_add(qq[:, sl], psB[:, sl], bq[:])
                for c in range(NCHUNK):
                    sl = slice(512 * c, 512 * (c + 1))
                    nc.vector.tensor_scalar_add(vk[:, sl], psA[:, sl], bkv[:])

            # ---------------- attention ----------------
            with tc.tile_pool(name="pv", bufs=1, space="PSUM") as pv, \
                 tc.tile_pool(name="po", bufs=1, space="PSUM") as po, \
                 tc.tile_pool(name="pst", bufs=3, space="PSUM") as pst, \
                 tc.tile_pool(name="et", bufs=3) as etpool:

                ops = [
                    po.tile([65, 512], f32, tag=f"o{j}", name=f"o{j}")
                    for j in range(NCHUNK)
                ]

                def emit_transposes(g):
                    # 4 transposes per group into one psum tile, then copy out
                    vt = pv.tile([128, 256], bf16, tag="vt")
                    for c in range(4):
                        i = 4 * g + c
                        nc.tensor.transpose(
                            vt[:, 64 * c:64 * (c + 1)],
                            vk[64:128, 128 * i:128 * (i + 1)], ident64[:],
                        )
                    for c in range(4):
                        i = 4 * g + c
                        nc.vector.tensor_copy(
                            v1[:, 65 * i:65 * i + 64], vt[:, 64 * c:64 * (c + 1)]
                        )

                ets = [None] * NS

                def emit_scores(i):
                    # S.T[s, t] for t in [ts, 2048), trimmed to causal boundary
                    ts = 128 * i
                    et = etpool.tile([128, T], bf16, tag="et")
                    ets[i] = et
                    for cc in range(ts // 512, NCHUNK):
                        t0 = 512 * cc
                        lo = max(t0, ts)
                        st = pst.tile([128, 512], f32, tag="st")
                        nc.tensor.matmul(
                            st[:, lo - t0:512],
                            vk[0:64, 128 * i:128 * (i + 1)],
                            qq[:, lo:t0 + 512],
                            start=True, stop=True,
                        )
                        nc.scalar.activation(
                            et[:, lo:t0 + 512],
                            st[:, lo - t0:512],
                            EXP, scale=SCALE,
                        )
                    # causal mask on the diagonal 128-block (gpsimd: SBUF-only)
                    nc.gpsimd.tensor_mul(
                        et[:, ts:ts + 128], et[:, ts:ts + 128], tri[:]
                    )

                def emit_pv(i):
                    ts = 128 * i
                    jmin = i // 4
                    et = ets[i]
                    for j in range(jmin, NCHUNK):
                        lo = max(512 * j, ts)
                        nc.tensor.matmul(
                            ops[j][:, lo - 512 * j:512],
                            v1[:, 65 * i:65 * i + 65],
                            et[:, lo:512 * (j + 1)],
                            start=(i == 0), stop=(i == 4 * j + 3),
                        )
                    # drain any output chunk whose accumulation just finished
                    for j in range(jmin, NCHUNK):
                        if i == 4 * j + 3:
                            sl = slice(512 * j, 512 * (j + 1))
                            nc.vector.tensor_copy(osb[:, sl], ops[j][:])
                            nc.sync.dma_start(out=out_d[:, sl], in_=osb[:, sl])

                emit_scores(0)
                for i in range(NS):
                    if i < 4:
                        emit_transposes(i)
                    if i < NS - 1:
                        emit_scores(i + 1)
                    emit_pv(i)

    nc.compile()
    return nc


def _get_nc():
    if "nc" not in _CACHE:
        _CACHE["nc"] = _build()
    return _CACHE["nc"]


def make_in_maps(X, Wq, bq, Wk, bk, Wv, bv):
    X = np.asarray(X, dtype=np.float32)
    Wq = np.asarray(Wq, dtype=np.float32)
    Wk = np.asarray(Wk, dtype=np.float32)
    Wv = np.asarray(Wv, dtype=np.float32)
    bq = np.asarray(bq, dtype=np.float32)
    bk = np.asarray(bk, dtype=np.float32)
    bv = np.asarray(bv, dtype=np.float32)

    wkv = np.ascontiguousarray(
        np.concatenate([Wk, Wv], axis=1).reshape(KT, 128, 128)
        .transpose(1, 0, 2).reshape(128, KT * 128)
    ).astype(BF16)
    wq = np.ascontiguousarray(
        Wq.reshape(KT, 128, 64).transpose(1, 0, 2).reshape(128, KT * 64)
    ).astype(BF16)
    bkv = np.concatenate([bk, bv]).reshape(128, 1).astype(np.float32)
    bqv = bq.reshape(64, 1).astype(np.float32)

    cst = np.zeros((128, 192), dtype=np.float32)
    cst[:, 0:128] = np.triu(np.ones((128, 128), dtype=np.float32))
    cst[64:128, 128:192] = np.eye(64, dtype=np.float32)
    cst = cst.astype(BF16)

    in_maps = []
    for b in range(B):
        xt = np.ascontiguousarray(X[b].T).reshape(KT, 128, T).astype(BF16)
        in_maps.append(
            {"xt": xt, "wkv": wkv, "wq": wq, "bkv": bkv, "bq": bqv,
             "cst": cst}
        )
    return in_maps


def kernel(X, Wq, bq, Wk, bk, Wv, bv):
    from concourse.bass_utils import run_bass_kernel_spmd

    nc = _get_nc()
    in_maps = make_in_maps(X, Wq, bq, Wk, bk, Wv, bv)
    res = run_bass_kernel_spmd(nc, in_maps, list(range(B)))

    out = np.empty((B, T, DK), dtype=np.float32)
    for b in range(B):
        r = res.results[b]["out"]
        out[b] = (r[:64] / r[64:65]).T
    return out


# revision 13
# speedup vs baseline: 1.5344x; 1.0627x over previous
"""Single-head causal self-attention on 8 NeuronCores (data-parallel over batch).

Reference computation (per batch element b):
    Q = X @ Wq + bq; K = X @ Wk + bk; V = X @ Wv + bv        # [T, DK]
    S = Q @ K.T / sqrt(DK)  (causal masked)
    out = softmax(S) @ V                                      # [T, DK]

Device strategy (one batch element per core), v2 (bf16):
  - Host passes X.T tiles and packed weights in bf16 (halves HBM traffic
    and SBUF pressure; PSUM accumulation stays fp32).
  - Pass A stationary packs [Wk | Wv] per 128-row C-chunk, so the psum
    holds K.T in partitions 0:64 and V.T in partitions 64:128. Pass B is
    just Wq -> Q.T in a [64, T] psum (no duplication).
  - X tiles stream over three DMA queues in k order; weights/constants go
    on the vector engine's queue so x0 is never stuck behind them.
  - k=0..6 accumulate; the k=7 chunk matmuls are emitted per 512-column
    chunk with the drain of that chunk right behind them, split across
    vector (vk even chunks), gpsimd (vk odd chunks), and scalar (qq with
    exact bias via Identity activation).
  - V.T tiles are PE-transposed into [s, dk] with a ones column appended
    (l = softmax denominator falls out of the output matmul, row 64).
  - Scores are computed transposed, S.T[s, t] = K.T^T @ Q.T, trimmed to
    the causal boundary at 128-column granularity; exp runs on scalar
    (scale=1/8 fused) writing bf16; causality = one triangular mask
    multiply per diagonal block (vector) + trimmed matmuls (no memsets).
  - The P@V contraction accumulates the four 512-wide output chunks in
    psum; per-chunk drain + output DMA as soon as its accumulation ends.
  - PE stream is software-pipelined: scores(i+1) is emitted before PV(i)
    so exp(i) overlaps matmuls; V transposes fill the drain window.
  - Device output per core: [65, T] = rows 0:64 unnormalized O.T, row 64
    l. Host computes (O_unnorm / l).T.
"""

import sys

sys.path.insert(0, "/opt/trn_rl_repo")

import numpy as np
import ml_dtypes

B, T, C, DK = 8, 2048, 1024, 64
KT = C // 128          # 8 k-tiles in the contraction over C
NS = T // 128          # 16 s-tiles (key blocks)
NCHUNK = T // 512      # 4 output chunks of 512
SCALE = 1.0 / np.sqrt(DK)
BF16 = ml_dtypes.bfloat16

_CACHE = {}


def _build():
    from concourse import bass, bacc, tile

    mybir = bass.mybir
    f32 = mybir.dt.float32
    bf16 = mybir.dt.bfloat16

    nc = bacc.Bacc(
        "TRN2", target_bir_lowering=False, debug=False, num_devices=B
    )

    xt_d = nc.dram_tensor("xt", [KT, 128, T], bf16, kind="ExternalInput")
    wkv_d = nc.dram_tensor("wkv", [128, KT * 128], bf16, kind="ExternalInput")
    wq_d = nc.dram_tensor("wq", [128, KT * 64], bf16, kind="ExternalInput")
    bkv_d = nc.dram_tensor("bkv", [128, 1], f32, kind="ExternalInput")
    bq_d = nc.dram_tensor("bq", [64, 1], f32, kind="ExternalInput")
    # cols 0:128 upper-tri mask; cols 128:192 identity (rows 64:128)
    cst_d = nc.dram_tensor("cst", [128, 192], bf16, kind="ExternalInput")
    out_d = nc.dram_tensor("out", [65, T], f32, kind="ExternalOutput")

    EXP = mybir.ActivationFunctionType.Exp
    IDENT = mybir.ActivationFunctionType.Identity

    with tile.TileContext(nc) as tc:
        with tc.tile_pool(name="const", bufs=1) as cpool, \
             tc.tile_pool(name="weights", bufs=1) as wpool, \
             tc.tile_pool(name="x", bufs=1) as xpool, \
             tc.tile_pool(name="acts", bufs=1) as apool:

            # --- inbound DMAs: weights/consts on vector's queue, X tiles
            # --- striped over sync/gpsimd/scalar in k order
            wkv = wpool.tile([128, KT * 128], bf16)
            wq = wpool.tile([128, KT * 64], bf16)
            cst = cpool.tile([128, 192], bf16)
            bkv = cpool.tile([128, 1], f32)
            bq = cpool.tile([64, 1], f32)
            tri = cst[:, 0:128]
            ident64 = cst[64:128, 128:192]

            xts = []
            for k in range(KT):
                xk = xpool.tile([128, T], bf16, tag=f"x{k}")
                xts.append(xk)

            # k=0 weight slices first (tiny), then x0 split sync+scalar so
            # k=0 starts ASAP; remaining tiles striped in k order
            nc.scalar.dma_start(out=wkv[:, 0:128], in_=wkv_d[:, 0:128])
            nc.scalar.dma_start(out=wq[:, 0:64], in_=wq_d[:, 0:64])
            nc.sync.dma_start(out=xts[0][:, 0:1024], in_=xt_d[0][:, 0:1024])
            nc.scalar.dma_start(out=xts[0][:, 1024:T], in_=xt_d[0][:, 1024:T])
            nc.scalar.dma_start(out=wkv[:, 128:KT * 128], in_=wkv_d[:, 128:KT * 128])
            nc.scalar.dma_start(out=wq[:, 64:KT * 64], in_=wq_d[:, 64:KT * 64])
            nc.scalar.dma_start(out=cst[:], in_=cst_d[:])
            nc.scalar.dma_start(out=bkv[:], in_=bkv_d[:])
            nc.scalar.dma_start(out=bq[:], in_=bq_d[:])
            xdma = {1: nc.sync, 2: nc.gpsimd, 3: nc.scalar, 4: nc.sync,
                    5: nc.gpsimd, 6: nc.scalar, 7: nc.sync}
            for k in range(1, KT):
                xdma[k].dma_start(out=xts[k][:], in_=xt_d[k])

            # persistent activations
            vk = apool.tile([128, T], bf16, tag="vk")   # K.T rows 0:64, V.T rows 64:128
            qq = apool.tile([64, T], bf16, tag="qq")    # Q.T
            v1 = apool.tile([128, NS * 65], bf16, tag="v1")  # [V_i | 1] stationaries
            osb = apool.tile([65, T], f32, tag="osb")

            nc.gpsimd.memset(v1[:], 1.0)

            # ---------------- projections ----------------
            with tc.tile_pool(name="pproj", bufs=1, space="PSUM") as pproj:
                psA = pproj.tile([128, T], f32, tag="psA")
                psB = pproj.tile([64, T], f32, tag="psB")
                for k in range(KT - 1):
                    for c in range(NCHUNK):
                        sl = slice(512 * c, 512 * (c + 1))
                        nc.tensor.matmul(
                            psA[:, sl],
                            wkv[:, 128 * k:128 * (k + 1)],
                            xts[k][:, sl],
                            start=(k == 0), stop=False,
                        )
                    for c in range(NCHUNK):
                        sl = slice(512 * c, 512 * (c + 1))
                        nc.tensor.matmul(
                            psB[:, sl],
                            wq[:, 64 * k:64 * (k + 1)],
                            xts[k][:, sl],
                            start=(k == 0), stop=False,
                        )
                # k=7 per chunk; drains right behind each chunk, interleaved
                # so vector delivers vk c0 first (gates transposes) and
                # scalar delivers qq c0 first (gates scores(0))
                k = KT - 1
                for c in range(NCHUNK):
                    sl = slice(512 * c, 512 * (c + 1))
                    nc.tensor.matmul(
                        psA[:, sl], wkv[:, 128 * k:128 * (k + 1)],
                        xts[k][:, sl], start=False, stop=True,
                    )
                    nc.tensor.matmul(
                        psB[:, sl], wq[:, 64 * k:64 * (k + 1)],
                        xts[k][:, sl], start=False, stop=True,
                    )
                    if c % 2 == 0:
                        nc.scalar.activation(
                            qq[:, sl], psB[:, sl], IDENT, bias=bq[:]
                        )
                        nc.vector.tensor_scalar_add(vk[:, sl], psA[:, sl], bkv[:])
                    else:
                        nc.vector.tensor_scalar_add(qq[:, sl], psB[:, sl], bq[:])
                        nc.scalar.activation(
                            vk[:, sl], psA[:, sl], IDENT, bias=bkv[:]
                        )

            # ---------------- V transposes ----------------
            # scoped psum pool between projections and attention so the
            # attention pools get the full 8 banks
            with tc.tile_pool(name="pv", bufs=2, space="PSUM") as pv:
                for g in range(4):
                    vt = pv.tile([128, 256], bf16, tag="vt")
                    for c in range(4):
                        i = 4 * g + c
                        nc.tensor.transpose(
                            vt[:, 64 * c:64 * (c + 1)],
                            vk[64:128, 128 * i:128 * (i + 1)], ident64[:],
                        )
                    for c in range(4):
                        i = 4 * g + c
                        nc.vector.tensor_copy(
                            v1[:, 65 * i:65 * i + 64], vt[:, 64 * c:64 * (c + 1)]
                        )

            # ---------------- attention ----------------
            with tc.tile_pool(name="po", bufs=1, space="PSUM") as po, \
                 tc.tile_pool(name="pst", bufs=2, space="PSUM") as pst, \
                 tc.tile_pool(name="et", bufs=3) as etpool:

                ops = [
                    po.tile([65, 512], f32, tag=f"o{j}", name=f"o{j}")
                    for j in range(NCHUNK)
                ]

                ets = [None] * NS

                def emit_scores(i):
                    # S.T[s, t] for t in [ts, 2048), trimmed to causal boundary
                    ts = 128 * i
                    et = etpool.tile([128, T], bf16, tag="et")
                    ets[i] = et
                    for tb in range(ts // 1024, 2):
                        st = pst.tile([128, 1024], f32, tag="st")
                        for cc in range(2):
                            t0 = 1024 * tb + 512 * cc
                            lo = max(t0, ts)
                            if t0 + 512 <= lo:
                                continue
                            nc.tensor.matmul(
                                st[:, lo - 1024 * tb:t0 + 512 - 1024 * tb],
                                vk[0:64, 128 * i:128 * (i + 1)],
                                qq[:, lo:t0 + 512],
                                start=True, stop=True,
                            )
                        off = max(0, ts - 1024 * tb)
                        nc.scalar.activation(
                            et[:, 1024 * tb + off:1024 * (tb + 1)],
                            st[:, off:1024],
                            EXP, scale=SCALE,
                        )
                    # causal mask on the diagonal 128-block (gpsimd: SBUF-only)
                    nc.gpsimd.tensor_mul(
                        et[:, ts:ts + 128], et[:, ts:ts + 128], tri[:]
                    )

                def emit_pv(i):
                    ts = 128 * i
                    jmin = i // 4
                    et = ets[i]
                    for j in range(jmin, NCHUNK):
                        lo = max(512 * j, ts)
                        nc.tensor.matmul(
                            ops[j][:, lo - 512 * j:512],
                            v1[:, 65 * i:65 * i + 65],
                            et[:, lo:512 * (j + 1)],
                            start=(i == 0), stop=(i == 4 * j + 3),
                        )
                    # drain any output chunk whose accumulation just finished
                    for j in range(jmin, NCHUNK):
                        if i == 4 * j + 3:
                            sl = slice(512 * j, 512 * (j + 1))
                            nc.vector.tensor_copy(osb[:, sl], ops[j][:])
                            nc.sync.dma_start(out=out_d[:, sl], in_=osb[:, sl])

                emit_scores(0)
                for i in range(NS):
                    if i < NS - 1:
                        emit_scores(i + 1)
                    emit_pv(i)

    nc.compile()
    return nc


def _get_nc():
    if "nc" not in _CACHE:
        _CACHE["nc"] = _build()
    return _CACHE["nc"]


def make_in_maps(X, Wq, bq, Wk, bk, Wv, bv):
    X = np.asarray(X, dtype=np.float32)
    Wq = np.asarray(Wq, dtype=np.float32)
    Wk = np.asarray(Wk, dtype=np.float32)
    Wv = np.asarray(Wv, dtype=np.float32)
    bq = np.asarray(bq, dtype=np.float32)
    bk = np.asarray(bk, dtype=np.float32)
    bv = np.asarray(bv, dtype=np.float32)

    wkv = np.ascontiguousarray(
        np.concatenate([Wk, Wv], axis=1).reshape(KT, 128, 128)
        .transpose(1, 0, 2).reshape(128, KT * 128)
    ).astype(BF16)
    wq = np.ascontiguousarray(
        Wq.reshape(KT, 128, 64).transpose(1, 0, 2).reshape(128, KT * 64)
    ).astype(BF16)
    bkv = np.concatenate([bk, bv]).reshape(128, 1).astype(np.float32)
    bqv = bq.reshape(64, 1).astype(np.float32)

    cst = np.zeros((128, 192), dtype=np.float32)
    cst[:, 0:128] = np.triu(np.ones((128, 128), dtype=np.float32))
    cst[64:128, 128:192] = np.eye(64, dtype=np.float32)
    cst = cst.astype(BF16)

    in_maps = []
    for b in range(B):
        xt = np.ascontiguousarray(X[b].T).reshape(KT, 128, T).astype(BF16)
        in_maps.append(
            {"xt": xt, "wkv": wkv, "wq": wq, "bkv": bkv, "bq": bqv,
             "cst": cst}
        )
    return in_maps


def kernel(X, Wq, bq, Wk, bk, Wv, bv):
    from concourse.bass_utils import run_bass_kernel_spmd

    nc = _get_nc()
    in_maps = make_in_maps(X, Wq, bq, Wk, bk, Wv, bv)
    res = run_bass_kernel_spmd(nc, in_maps, list(range(B)))

    out = np.empty((B, T, DK), dtype=np.float32)
    for b in range(B):
        r = res.results[b]["out"]
        out[b] = (r[:64] / r[64:65]).T
    return out


# revision 17
# speedup vs baseline: 1.7434x; 1.1362x over previous
"""Single-head causal self-attention on 8 NeuronCores (data-parallel over batch).

Reference computation (per batch element b):
    Q = X @ Wq + bq; K = X @ Wk + bk; V = X @ Wv + bv        # [T, DK]
    S = Q @ K.T / sqrt(DK)  (causal masked)
    out = softmax(S) @ V                                      # [T, DK]

Device strategy (one batch element per core), v2 (bf16):
  - Host passes X.T tiles and packed weights in bf16 (halves HBM traffic
    and SBUF pressure; PSUM accumulation stays fp32).
  - Pass A stationary packs [Wk | Wv] per 128-row C-chunk, so the psum
    holds K.T in partitions 0:64 and V.T in partitions 64:128. Pass B is
    just Wq -> Q.T in a [64, T] psum (no duplication).
  - X tiles stream over three DMA queues in k order; weights/constants go
    on the vector engine's queue so x0 is never stuck behind them.
  - k=0..6 accumulate; the k=7 chunk matmuls are emitted per 512-column
    chunk with the drain of that chunk right behind them, split across
    vector (vk even chunks), gpsimd (vk odd chunks), and scalar (qq with
    exact bias via Identity activation).
  - V.T tiles are PE-transposed into [s, dk] with a ones column appended
    (l = softmax denominator falls out of the output matmul, row 64).
  - Scores are computed transposed, S.T[s, t] = K.T^T @ Q.T, trimmed to
    the causal boundary at 128-column granularity; exp runs on scalar
    (scale=1/8 fused) writing bf16; causality = one triangular mask
    multiply per diagonal block (vector) + trimmed matmuls (no memsets).
  - The P@V contraction accumulates the four 512-wide output chunks in
    psum; per-chunk drain + output DMA as soon as its accumulation ends.
  - PE stream is software-pipelined: scores(i+1) is emitted before PV(i)
    so exp(i) overlaps matmuls; V transposes fill the drain window.
  - Device output per core: [65, T] = rows 0:64 unnormalized O.T, row 64
    l. Host computes (O_unnorm / l).T.
"""

import sys

sys.path.insert(0, "/opt/trn_rl_repo")

import numpy as np
import ml_dtypes

B, T, C, DK = 8, 2048, 1024, 64
KT = C // 128          # 8 k-tiles in the contraction over C
NS = T // 128          # 16 s-tiles (key blocks)
NCHUNK = T // 512      # 4 output chunks of 512
SCALE = 1.0 / np.sqrt(DK)
BF16 = ml_dtypes.bfloat16

_CACHE = {}


def _build():
    from concourse import bass, bacc, tile

    mybir = bass.mybir
    f32 = mybir.dt.float32
    bf16 = mybir.dt.bfloat16

    nc = bacc.Bacc(
        "TRN2", target_bir_lowering=False, debug=False, num_devices=B
    )

    xt_d = nc.dram_tensor("xt", [KT, 128, T], bf16, kind="ExternalInput")
    wkv_d = nc.dram_tensor("wkv", [128, KT * 128], bf16, kind="ExternalInput")
    wq_d = nc.dram_tensor("wq", [128, KT * 64], bf16, kind="ExternalInput")
    bkv_d = nc.dram_tensor("bkv", [128, 1], f32, kind="ExternalInput")
    bq_d = nc.dram_tensor("bq", [64, 1], f32, kind="ExternalInput")
    # cols 0:128 upper-tri mask; cols 128:192 identity (rows 64:128)
    cst_d = nc.dram_tensor("cst", [128, 192], bf16, kind="ExternalInput")
    out_d = nc.dram_tensor("out", [65, T], f32, kind="ExternalOutput")

    EXP = mybir.ActivationFunctionType.Exp
    IDENT = mybir.ActivationFunctionType.Identity

    with tile.TileContext(nc) as tc:
        with tc.tile_pool(name="const", bufs=1) as cpool, \
             tc.tile_pool(name="weights", bufs=1) as wpool, \
             tc.tile_pool(name="x", bufs=1) as xpool, \
             tc.tile_pool(name="acts", bufs=1) as apool:

            # --- inbound DMAs: weights/consts on vector's queue, X tiles
            # --- striped over sync/gpsimd/scalar in k order
            wkv = wpool.tile([128, KT * 128], bf16)
            wq = wpool.tile([128, KT * 64], bf16)
            cst = cpool.tile([128, 192], bf16)
            bkv = cpool.tile([128, 1], f32)
            bq = cpool.tile([64, 1], f32)
            tri = cst[:, 0:128]
            ident64 = cst[64:128, 128:192]

            xts = []
            for k in range(KT):
                xk = xpool.tile([128, T], bf16, tag=f"x{k}")
                xts.append(xk)

            # k=0 weight slices first (tiny), x0 split across the two
            # fastest queues; remaining tiles striped in k order weighted
            # by measured queue speed (scalar > gpsimd > sync)
            nc.scalar.dma_start(out=wkv[:, 0:128], in_=wkv_d[:, 0:128])
            nc.scalar.dma_start(out=wq[:, 0:64], in_=wq_d[:, 0:64])
            nc.scalar.dma_start(out=xts[0][:, 0:1024], in_=xt_d[0][:, 0:1024])
            nc.gpsimd.dma_start(out=xts[0][:, 1024:T], in_=xt_d[0][:, 1024:T])
            nc.sync.dma_start(out=xts[1][:], in_=xt_d[1])
            nc.scalar.dma_start(out=wkv[:, 128:KT * 128], in_=wkv_d[:, 128:KT * 128])
            nc.scalar.dma_start(out=wq[:, 64:KT * 64], in_=wq_d[:, 64:KT * 64])
            nc.scalar.dma_start(out=cst[:], in_=cst_d[:])
            nc.scalar.dma_start(out=bkv[:], in_=bkv_d[:])
            nc.scalar.dma_start(out=bq[:], in_=bq_d[:])
            xdma = {2: nc.gpsimd, 3: nc.scalar, 4: nc.sync,
                    5: nc.gpsimd, 6: nc.scalar, 7: nc.gpsimd}
            for k in range(2, KT):
                xdma[k].dma_start(out=xts[k][:], in_=xt_d[k])

            # persistent activations
            vk = apool.tile([128, T], bf16, tag="vk")   # K.T rows 0:64, V.T rows 64:128
            qq = apool.tile([64, T], bf16, tag="qq")    # Q.T
            v1 = apool.tile([128, NS * 65], bf16, tag="v1")  # [V_i | 1] stationaries
            osb = apool.tile([65, T], f32, tag="osb")

            nc.gpsimd.memset(v1[:], 1.0)

            # ---------------- projections ----------------
            with tc.tile_pool(name="pproj", bufs=1, space="PSUM") as pproj:
                psA = pproj.tile([128, T], f32, tag="psA")
                psB = pproj.tile([64, T], f32, tag="psB")
                for k in range(KT - 1):
                    for c in range(NCHUNK):
                        sl = slice(512 * c, 512 * (c + 1))
                        nc.tensor.matmul(
                            psA[:, sl],
                            wkv[:, 128 * k:128 * (k + 1)],
                            xts[k][:, sl],
                            start=(k == 0), stop=False,
                        )
                    for c in range(NCHUNK):
                        sl = slice(512 * c, 512 * (c + 1))
                        nc.tensor.matmul(
                            psB[:, sl],
                            wq[:, 64 * k:64 * (k + 1)],
                            xts[k][:, sl],
                            start=(k == 0), stop=False,
                        )
                # k=7 per chunk; drains right behind each chunk, interleaved
                # so vector delivers vk c0 first (gates transposes) and
                # scalar delivers qq c0 first (gates scores(0))
                k = KT - 1
                for c in range(NCHUNK):
                    sl = slice(512 * c, 512 * (c + 1))
                    nc.tensor.matmul(
                        psA[:, sl], wkv[:, 128 * k:128 * (k + 1)],
                        xts[k][:, sl], start=False, stop=True,
                    )
                    nc.tensor.matmul(
                        psB[:, sl], wq[:, 64 * k:64 * (k + 1)],
                        xts[k][:, sl], start=False, stop=True,
                    )
                    if c % 2 == 0:
                        nc.scalar.activation(
                            qq[:, sl], psB[:, sl], IDENT, bias=bq[:]
                        )
                        nc.vector.tensor_scalar_add(vk[:, sl], psA[:, sl], bkv[:])
                    else:
                        nc.vector.tensor_scalar_add(qq[:, sl], psB[:, sl], bq[:])
                        nc.scalar.activation(
                            vk[:, sl], psA[:, sl], IDENT, bias=bkv[:]
                        )

            # ---------------- V transposes ----------------
            # scoped psum pool between projections and attention so the
            # attention pools get the full 8 banks
            with tc.tile_pool(name="pv", bufs=2, space="PSUM") as pv:
                for g in range(4):
                    vt = pv.tile([128, 256], bf16, tag="vt")
                    for c in range(4):
                        i = 4 * g + c
                        nc.tensor.transpose(
                            vt[:, 64 * c:64 * (c + 1)],
                            vk[64:128, 128 * i:128 * (i + 1)], ident64[:],
                        )
                    for c in range(4):
                        i = 4 * g + c
                        nc.vector.tensor_copy(
                            v1[:, 65 * i:65 * i + 64], vt[:, 64 * c:64 * (c + 1)]
                        )

            # ---------------- attention ----------------
            with tc.tile_pool(name="po", bufs=1, space="PSUM") as po, \
                 tc.tile_pool(name="pst", bufs=2, space="PSUM") as pst, \
                 tc.tile_pool(name="et", bufs=3) as etpool:

                ops = [
                    po.tile([65, 512], f32, tag=f"o{j}", name=f"o{j}")
                    for j in range(NCHUNK)
                ]

                ets = [None] * NS

                def emit_scores(i):
                    # S.T[s, t] for t in [ts, 2048), trimmed to causal boundary
                    ts = 128 * i
                    et = etpool.tile([128, T], bf16, tag="et")
                    ets[i] = et
                    for tb in range(ts // 1024, 2):
                        st = pst.tile([128, 1024], f32, tag="st")
                        for cc in range(2):
                            t0 = 1024 * tb + 512 * cc
                            lo = max(t0, ts)
                            if t0 + 512 <= lo:
                                continue
                            nc.tensor.matmul(
                                st[:, lo - 1024 * tb:t0 + 512 - 1024 * tb],
                                vk[0:64, 128 * i:128 * (i + 1)],
                                qq[:, lo:t0 + 512],
                                start=True, stop=True,
                            )
                        off = max(0, ts - 1024 * tb)
                        nc.scalar.activation(
                            et[:, 1024 * tb + off:1024 * (tb + 1)],
                            st[:, off:1024],
                            EXP, scale=SCALE,
                        )
                    # causal mask on the diagonal 128-block (gpsimd: SBUF-only)
                    nc.gpsimd.tensor_mul(
                        et[:, ts:ts + 128], et[:, ts:ts + 128], tri[:]
                    )

                def emit_pv(i):
                    ts = 128 * i
                    jmin = i // 4
                    et = ets[i]
                    for j in range(jmin, NCHUNK):
                        lo = max(512 * j, ts)
                        nc.tensor.matmul(
                            ops[j][:, lo - 512 * j:512],
                            v1[:, 65 * i:65 * i + 65],
                            et[:, lo:512 * (j + 1)],
                            start=(i == 0), stop=(i == 4 * j + 3),
                        )
                    # drain any output chunk whose accumulation just finished
                    for j in range(jmin, NCHUNK):
                        if i == 4 * j + 3:
                            sl = slice(512 * j, 512 * (j + 1))
                            nc.vector.tensor_copy(osb[:, sl], ops[j][:])
                            nc.sync.dma_start(out=out_d[:, sl], in_=osb[:, sl])

                emit_scores(0)
                for i in range(NS):
                    if i < NS - 1:
                        emit_scores(i + 1)
                    emit_pv(i)

    nc.compile()
    return nc


def _get_nc():
    if "nc" not in _CACHE:
        _CACHE["nc"] = _build()
    return _CACHE["nc"]


def make_in_maps(X, Wq, bq, Wk, bk, Wv, bv):
    X = np.asarray(X, dtype=np.float32)
    Wq = np.asarray(Wq, dtype=np.float32)
    Wk = np.asarray(Wk, dtype=np.float32)
    Wv = np.asarray(Wv, dtype=np.float32)
    bq = np.asarray(bq, dtype=np.float32)
    bk = np.asarray(bk, dtype=np.float32)
    bv = np.asarray(bv, dtype=np.float32)

    wkv = np.ascontiguousarray(
        np.concatenate([Wk, Wv], axis=1).reshape(KT, 128, 128)
        .transpose(1, 0, 2).reshape(128, KT * 128)
    ).astype(BF16)
    wq = np.ascontiguousarray(
        Wq.reshape(KT, 128, 64).transpose(1, 0, 2).reshape(128, KT * 64)
    ).astype(BF16)
    bkv = np.concatenate([bk, bv]).reshape(128, 1).astype(np.float32)
    bqv = bq.reshape(64, 1).astype(np.float32)

    cst = np.zeros((128, 192), dtype=np.float32)
    cst[:, 0:128] = np.triu(np.ones((128, 128), dtype=np.float32))
    cst[64:128, 128:192] = np.eye(64, dtype=np.float32)
    cst = cst.astype(BF16)

    in_maps = []
    for b in range(B):
        xt = np.ascontiguousarray(X[b].T).reshape(KT, 128, T).astype(BF16)
        in_maps.append(
            {"xt": xt, "wkv": wkv, "wq": wq, "bkv": bkv, "bq": bqv,
             "cst": cst}
        )
    return in_maps


def kernel(X, Wq, bq, Wk, bk, Wv, bv):
    from concourse.bass_utils import run_bass_kernel_spmd

    nc = _get_nc()
    in_maps = make_in_maps(X, Wq, bq, Wk, bk, Wv, bv)
    res = run_bass_kernel_spmd(nc, in_maps, list(range(B)))

    out = np.empty((B, T, DK), dtype=np.float32)
    for b in range(B):
        r = res.results[b]["out"]
        out[b] = (r[:64] / r[64:65]).T
    return out
